# revision 36
# baseline (speedup 1.0000x reference)
"""CartesianMACE rank-0 fused kernel for 8 trn2 NeuronCores (v6).

Only the rank-0 path reaches the output (ranks 1/2 of the reference are
dead code), so per node n with 16x16 mats A=cw0[0,n], B=mw0[0,n],
D=cw1[0,n], E=mw1[0,n] and 16-vecs x=h0[n], m0=msg0_r0[n], m1=msg1_r0[n]:

    s[n] = colsum(D) . (A x + B m0) + colsum(E) . m1
    out  = [sum_n s[n] w_pred[0,n], sum_n s[n] w_pred[1,n]] + b_pred

Design (f32 baseline ~98us; v4 ~71us):
  * All streamed data bf16: halves HBM traffic (13.4MB/core) and gives
    DVE tensor_tensor the 2x perf mode (hardware-verified in traces).
  * DVE-ONLY compute. GpSimd and DVE arbitrate an exclusive lock on the
    shared SBUF port pair: a running GpSimd tensor op makes concurrent
    bf16 2x DVE ops 2.5-4x slower (measured), so GpSimd is kept idle.
  * Reductions are bf16 pairwise fold-trees (2x mode), not 1x
    reduce_sum. A-side products and D/E tiles fold into one shared t8
    tile so deeper levels cover both trees in single fat instructions.
  * The last fold level (stride-2 operands -> 1x mode) is skipped:
    pairs survive into the epilogue, where the cd*t dot product runs
    at doubled width in 2x mode instead (net win).
  * de lands in its own per-span tiles - sharing a tile between DMA
    writes and the mul's engine writes creates a false WAW dependency
    that stalls the ramp (cost ~4us in v4).
  * Nodes on SBUF partitions: 50000 padded to 50176 = 8 cores x 7
    supertiles x 128 partitions x 7 groups. Spans [1,2,2,2] supertiles;
    all DMAs HWDGE with one merged transfer per span half (ab layout
    [P, T*3584] so any slice is one descriptor per partition), ab0+xm
    first for a fast ramp, epilogue-only m1/w last. Per-core (128,2)
    partials are summed on host with b_pred (the head's all-reduce).
"""

import sys

for _p in ("/opt/trn_rl_repo", "/root/.axon_site/_ro/trn_rl_repo"):
    if _p not in sys.path:
        sys.path.append(_p)

import numpy as np
import ml_dtypes

BF16 = ml_dtypes.bfloat16

N, CH = 50000, 16
CORES = 8
T, S = 7, 7          # supertiles per core, groups per supertile
GP = T * S           # 49 groups of 128 nodes per core
NP = CORES * T * 128 * S  # 50176 padded nodes
SPANS = [(0, 1), (1, 2), (3, 2), (5, 2)]  # (first supertile, length)

_cache = {}
TRACE = False


def _split_multiwait(nc, mybir):
    """This walrus build accepts a single sync-wait per instruction, but Tile
    attaches one wait per producer proc. Split: keep the last wait on the
    instruction and hoist the rest onto fresh same-engine Drain carriers
    inserted immediately before it (engines execute their stream in-order,
    so semantics are identical)."""
    for fn in nc.m.functions:
        for bb in fn.blocks:
            insts = bb.instructions  # live list
            i = 0
            while i < len(insts):
                ins = insts[i]
                si = ins.sync_info
                if si is not None and len(si.on_wait) > 1:
                    waits = list(si.on_wait)
                    ins.sync_info = mybir.SyncInfo(
                        on_wait=waits[-1:], on_update=list(si.on_update))
                    for k, w in enumerate(waits[:-1]):
                        insts.insert(i + k, mybir.InstDrain(
                            name=f"{ins.name}_w{k}", opcode="Drain",
                            engine=ins.engine, ins=[], outs=[],
                            sync_info=mybir.SyncInfo(on_wait=[w], on_update=[]),
                        ))
                    i += len(waits) - 1
                i += 1


def _build_nc():
    import concourse.bass as bass
    import concourse.tile as tile
    import concourse.mybir as mybir

    f32 = mybir.dt.float32
    b16 = mybir.dt.bfloat16
    P = 128

    nc = bass.Bass("TRN2", target_bir_lowering=False, debug=False,
                   num_devices=CORES)

    ab_d = nc.dram_tensor("ab", [P, T * 3584], b16, kind="ExternalInput").ap()
    de_d = nc.dram_tensor("de", [P, T * 3584], b16, kind="ExternalInput").ap()
    xm_d = nc.dram_tensor("xm", [P, T * 224], b16, kind="ExternalInput").ap()
    m1_d = nc.dram_tensor("m1", [P, T * 112], b16, kind="ExternalInput").ap()
    w_d = nc.dram_tensor("w", [P, 2 * GP], b16, kind="ExternalInput").ap()
    o_d = nc.dram_tensor("o", [P, 2], f32, kind="ExternalOutput").ap()

    F2R = 2 * T * 224  # 3136: [t, m, g, k, r2] per-partition layout
    F1 = T * 224       # 1568

    with tile.TileContext(nc) as tc:
        with (
            tc.tile_pool(name="big", bufs=1) as big,
            tc.tile_pool(name="work", bufs=1) as work,
        ):
            ab_all = big.tile([P, T * 3584], b16)
            xm_sb = big.tile([P, T * 224], b16)
            w_sb = big.tile([P, 2 * GP], b16)
            # ct2[:, 0:3136] = cd pairs (D|E colsums), [:, 3136:] = t pairs
            ct2 = big.tile([P, 2 * F2R], b16)
            vv = big.tile([P, F1], b16)     # [t, sel, g, k]: tn | m1
            cdf = big.tile([P, F1], b16)    # [t, m, g, k]: cd | ce
            pr = big.tile([P, F1], b16)
            tn2 = big.tile([P, F1], b16)    # m-summed t pairs

            des = []
            for si, (t0, ts) in enumerate(SPANS):
                E0, EN = t0 * 3584, ts * 3584
                nc.sync.dma_start(out=ab_all[:, E0:E0 + EN],
                                  in_=ab_d[:, E0:E0 + EN])
                if si == 0:
                    # xm rides right after ab0 so the first mul starts early
                    nc.sync.dma_start(out=xm_sb[:, :], in_=xm_d)
                de = work.tile([P, 2 * 3584], b16, tag="de", bufs=3)
                des.append(de)
                nc.sync.dma_start(out=de[:, 0:EN], in_=de_d[:, E0:E0 + EN])
            # epilogue-only inputs last: m1 into its vv slot (sel=1), w
            nc.sync.dma_start(
                out=vv[:, :].rearrange("p (t s x) -> p t s x",
                                       t=T, s=2, x=112)[:, :, 1],
                in_=m1_d.rearrange("p (t x) -> p t x", t=T, x=112))
            nc.sync.dma_start(out=w_sb[:, :], in_=w_d)

            t4 = work.tile([P, 2 * 1792], b16)
            for si, (t0, ts) in enumerate(SPANS):
                de = des[si]
                EN = ts * 3584
                U = ts * 14          # (t, m, g) groups in span
                tmp = work.tile([P, 2 * 3584], b16, tag="tmp", bufs=2)
                t8 = work.tile([P, 2 * 3584], b16, tag="t8", bufs=2)
                gjk = lambda ap: ap.rearrange("p (u j k) -> p u j k",
                                              u=U, j=16, k=16)
                xm_bc = (xm_sb[:, t0 * 224:(t0 + ts) * 224]
                         .rearrange("p (u k) -> p u k", u=U, k=16)
                         .unsqueeze(2).broadcast_to((P, U, 16, 16)))
                nc.vector.tensor_mul(out=gjk(tmp[:, 0:EN]),
                                     in0=gjk(ab_all[:, t0 * 3584:
                                                    (t0 + ts) * 3584]),
                                     in1=xm_bc)
                # fold 16->8: de half and tmp half -> adjacent halves of t8
                HV = ts * 224        # 16-wide rows per half
                d16 = de[:, 0:EN].rearrange("p (v k) -> p v k", v=HV, k=16)
                a16 = tmp[:, 0:EN].rearrange("p (v k) -> p v k", v=HV, k=16)
                e8d = t8[:, 0:HV * 8].rearrange("p (v k) -> p v k",
                                                v=HV, k=8)
                e8a = t8[:, HV * 8:HV * 16].rearrange("p (v k) -> p v k",
                                                      v=HV, k=8)
                nc.vector.tensor_add(out=e8d, in0=d16[:, :, 0:8],
                                     in1=d16[:, :, 8:16])
                nc.vector.tensor_add(out=e8a, in0=a16[:, :, 0:8],
                                     in1=a16[:, :, 8:16])
                # merged 8->4 over [de | tmp]
                V = ts * 448
                e8 = t8[:, 0:V * 8].rearrange("p (v k) -> p v k", v=V, k=8)
                e4 = t4[:, 0:V * 4].rearrange("p (v k) -> p v k", v=V, k=4)
                nc.vector.tensor_add(out=e4, in0=e8[:, :, 0:4],
                                     in1=e8[:, :, 4:8])
                # merged 4->2, pairs kept: -> two segments of ct2
                ct_v = (ct2[:, :].rearrange("p (c f) -> p c f", c=2, f=F2R)
                        [:, :, t0 * 448:(t0 + ts) * 448]
                        .rearrange("p c (y r) -> p c y r",
                                   y=ts * 224, r=2))
                nc.vector.tensor_add(
                    out=ct_v,
                    in0=e4[:, :, 0:2].rearrange("p (c y) r -> p c y r",
                                                c=2, y=ts * 224),
                    in1=e4[:, :, 2:4].rearrange("p (c y) r -> p c y r",
                                                c=2, y=ts * 224))

            # ---- epilogue ----
            cd2 = ct2[:, 0:F2R]                  # [t, m, g, k, r] pairs
            t12 = ct2[:, F2R:2 * F2R]
            tmx = lambda ap: ap.rearrange("p (t m x) -> p t m x",
                                          t=T, m=2, x=224)
            # tn2[t,g,k,r] = t12[t,0,..] + t12[t,1,..]   (m-sum, 2x)
            tn2h = tn2[:, 0:F1].rearrange("p (t x) -> p t x", t=T, x=224)
            nc.vector.tensor_add(out=tn2h, in0=tmx(t12)[:, :, 0],
                                 in1=tmx(t12)[:, :, 1])
            # collapse pairs (1x, small): tn -> vv[sel=0]; cd2 -> cdf
            tr = tn2[:, 0:F1].rearrange("p (v r) -> p v r", v=F1 // 2, r=2)
            nc.vector.tensor_add(
                out=vv[:, :].rearrange("p (t s x) -> p t s x",
                                       t=T, s=2, x=112)[:, :, 0],
                in0=tr[:, :, 0].rearrange("p (t x) -> p t x", t=T, x=112),
                in1=tr[:, :, 1].rearrange("p (t x) -> p t x", t=T, x=112))
            cr = cd2.rearrange("p (v r) -> p v r", v=F2R // 2, r=2)
            nc.vector.tensor_add(out=cdf[:, :], in0=cr[:, :, 0],
                                 in1=cr[:, :, 1])
            # pr[t,m,g,k] = cdf * (tn | m1)
            nc.vector.tensor_mul(out=pr[:, :], in0=cdf[:, :], in1=vv[:, :])
            # fold [98, 16] -> [98]
            p16 = pr[:, :].rearrange("p (v k) -> p v k", v=98, k=16)
            h8 = tn2[:, 0:784].rearrange("p (v k) -> p v k", v=98, k=8)
            nc.vector.tensor_add(out=h8, in0=p16[:, :, 0:8],
                                 in1=p16[:, :, 8:16])
            h4 = tn2[:, 784:1176].rearrange("p (v k) -> p v k", v=98, k=4)
            nc.vector.tensor_add(out=h4, in0=h8[:, :, 0:4], in1=h8[:, :, 4:8])
            h2 = tn2[:, 1176:1372].rearrange("p (v k) -> p v k", v=98, k=2)
            nc.vector.tensor_add(out=h2, in0=h4[:, :, 0:2], in1=h4[:, :, 2:4])
            h1 = tn2[:, 1372:1470]
            nc.vector.tensor_add(out=h1, in0=h2[:, :, 0], in1=h2[:, :, 1])
            # s[t,g] = h1[t,0,g] + h1[t,1,g]
            s_all = tn2[:, 1470:1519]
            gm = h1.rearrange("p (t m g) -> p t m g", t=T, m=2, g=S)
            nc.vector.tensor_add(out=s_all.rearrange("p (t g) -> p t g",
                                                     t=T, g=S),
                                 in0=gm[:, :, 0], in1=gm[:, :, 1])
            # head: hm[c, tg] = s[tg] * w[c, tg]; o[c] = sum_tg hm
            hm = pr[:, 0:2 * GP].rearrange("p (c q) -> p c q", c=2, q=GP)
            nc.vector.tensor_mul(
                out=hm,
                in0=w_sb[:, :].rearrange("p (c q) -> p c q", c=2, q=GP),
                in1=s_all.unsqueeze(1).broadcast_to((P, 2, GP)))
            o_sb = big.tile([P, 2], f32)
            nc.vector.reduce_sum(out=o_sb[:, :].rearrange("p c -> p c"),
                                 in_=hm, axis=mybir.AxisListType.X)
            nc.sync.dma_start(out=o_d, in_=o_sb[:, :])

    return nc


def _get_nc():
    if "nc" not in _cache:
        _cache["nc"] = _build_nc()
        import concourse.mybir as mybir
        _split_multiwait(_cache["nc"], mybir)
    return _cache["nc"]


def kernel(h0, cw0, mw0, cw1, mw1,
           msg0_r0, msg0_r1, msg0_r2,
           msg1_r0, msg1_r1, msg1_r2,
           w_pred, b_pred):
    from concourse.bass_utils import run_bass_kernel_spmd

    nc = _get_nc()

    def pad_mat(m):
        out = np.zeros((NP, 256), np.float32)
        out[:N] = np.asarray(m, np.float32).reshape(N, 256)
        return out.reshape(CORES, T, 128, S, 16, 16)  # [c,t,p,g,j,k]

    A5 = pad_mat(cw0[0])
    B5 = pad_mat(mw0[0])
    # AB: [c,t,p, m,g,j,k] -> (c,t,p,3584)
    AB = np.ascontiguousarray(
        np.stack([A5, B5], axis=3).reshape(CORES, T, 128, 3584)
        .transpose(0, 2, 1, 3).reshape(CORES, 128, T * 3584)).astype(BF16)

    D5 = pad_mat(cw1[0])
    E5 = pad_mat(mw1[0])
    # DE: j innermost for the fold tree: [c,t,p, m,g,k,j] -> (c,t,p,3584)
    DE = np.ascontiguousarray(
        np.stack([D5.transpose(0, 1, 2, 3, 5, 4),
                  E5.transpose(0, 1, 2, 3, 5, 4)], axis=3)
        .reshape(CORES, T, 128, 3584)
        .transpose(0, 2, 1, 3).reshape(CORES, 128, T * 3584)).astype(BF16)

    def pad_vec(v):
        out = np.zeros((NP, 16), np.float32)
        out[:N] = np.asarray(v, np.float32).reshape(N, 16)
        return out.reshape(CORES, T, 128, S, 16)

    X = pad_vec(np.asarray(h0, np.float32)[..., 0])
    M0 = pad_vec(np.asarray(msg0_r0, np.float32)[..., 0])
    XM = np.ascontiguousarray(
        np.stack([X, M0], axis=3).reshape(CORES, T, 128, 224)
        .transpose(0, 2, 1, 3).reshape(CORES, 128, T * 224)).astype(BF16)
    M1 = np.ascontiguousarray(
        pad_vec(np.asarray(msg1_r0, np.float32)[..., 0])
        .reshape(CORES, T, 128, 112)
        .transpose(0, 2, 1, 3).reshape(CORES, 128, T * 112)).astype(BF16)

    wp = np.zeros((2, NP), np.float32)
    wp[:, :N] = np.asarray(w_pred, np.float32)
    W = np.ascontiguousarray(
        wp.reshape(2, CORES, T, 128, S).transpose(1, 3, 0, 2, 4)
        .reshape(CORES, 128, 2 * GP)).astype(BF16)

    in_maps = [
        {"ab": AB[i], "de": DE[i], "xm": XM[i], "m1": M1[i], "w": W[i]}
        for i in range(CORES)
    ]
    res = run_bass_kernel_spmd(nc, in_maps, list(range(CORES)), trace=TRACE)
    _cache["last_res"] = res
    partial = np.zeros(2, np.float64)
    for i in range(CORES):
        partial += res.results[i]["o"].astype(np.float64).sum(axis=0)
    out = (partial + np.asarray(b_pred, np.float64)).astype(np.float32)
    return out.reshape(1, 2)


# revision 44
# speedup vs baseline: 1.0120x; 1.0120x over previous
"""CartesianMACE rank-0 fused kernel for 8 trn2 NeuronCores (v6).

Only the rank-0 path reaches the output (ranks 1/2 of the reference are
dead code), so per node n with 16x16 mats A=cw0[0,n], B=mw0[0,n],
D=cw1[0,n], E=mw1[0,n] and 16-vecs x=h0[n], m0=msg0_r0[n], m1=msg1_r0[n]:

    s[n] = colsum(D) . (A x + B m0) + colsum(E) . m1
    out  = [sum_n s[n] w_pred[0,n], sum_n s[n] w_pred[1,n]] + b_pred

Design (f32 baseline ~98us; v4 ~71us):
  * All streamed data bf16: halves HBM traffic (13.4MB/core) and gives
    DVE tensor_tensor the 2x perf mode (hardware-verified in traces).
  * DVE-ONLY compute. GpSimd and DVE arbitrate an exclusive lock on the
    shared SBUF port pair: a running GpSimd tensor op makes concurrent
    bf16 2x DVE ops 2.5-4x slower (measured), so GpSimd is kept idle.
  * Reductions are bf16 pairwise fold-trees (2x mode), not 1x
    reduce_sum. A-side products and D/E tiles fold into one shared t8
    tile so deeper levels cover both trees in single fat instructions.
  * The last fold level (stride-2 operands -> 1x mode) is skipped:
    pairs survive into the epilogue, where the cd*t dot product runs
    at doubled width in 2x mode instead (net win).
  * de lands in its own per-span tiles - sharing a tile between DMA
    writes and the mul's engine writes creates a false WAW dependency
    that stalls the ramp (cost ~4us in v4).
  * Nodes on SBUF partitions: 50000 padded to 50176 = 8 cores x 7
    supertiles x 128 partitions x 7 groups. Spans [1,2,2,2] supertiles;
    all DMAs HWDGE with one merged transfer per span half (ab layout
    [P, T*3584] so any slice is one descriptor per partition), ab0+xm
    first for a fast ramp, epilogue-only m1/w last. Per-core (128,2)
    partials are summed on host with b_pred (the head's all-reduce).
"""

import sys

for _p in ("/opt/trn_rl_repo", "/root/.axon_site/_ro/trn_rl_repo"):
    if _p not in sys.path:
        sys.path.append(_p)

import numpy as np
import ml_dtypes

BF16 = ml_dtypes.bfloat16

N, CH = 50000, 16
CORES = 8
T, S = 7, 7          # supertiles per core, groups per supertile
GP = T * S           # 49 groups of 128 nodes per core
NP = CORES * T * 128 * S  # 50176 padded nodes
SPANS = [(0, 1), (1, 2), (3, 2), (5, 2)]  # (first supertile, length)

_cache = {}
TRACE = False


def _split_multiwait(nc, mybir):
    """This walrus build accepts a single sync-wait per instruction, but Tile
    attaches one wait per producer proc. Split: keep the last wait on the
    instruction and hoist the rest onto fresh same-engine Drain carriers
    inserted immediately before it (engines execute their stream in-order,
    so semantics are identical)."""
    for fn in nc.m.functions:
        for bb in fn.blocks:
            insts = bb.instructions  # live list
            i = 0
            while i < len(insts):
                ins = insts[i]
                si = ins.sync_info
                if si is not None and len(si.on_wait) > 1:
                    waits = list(si.on_wait)
                    ins.sync_info = mybir.SyncInfo(
                        on_wait=waits[-1:], on_update=list(si.on_update))
                    for k, w in enumerate(waits[:-1]):
                        insts.insert(i + k, mybir.InstDrain(
                            name=f"{ins.name}_w{k}", opcode="Drain",
                            engine=ins.engine, ins=[], outs=[],
                            sync_info=mybir.SyncInfo(on_wait=[w], on_update=[]),
                        ))
                    i += len(waits) - 1
                i += 1


def _build_nc():
    import concourse.bass as bass
    import concourse.tile as tile
    import concourse.mybir as mybir

    f32 = mybir.dt.float32
    b16 = mybir.dt.bfloat16
    P = 128

    nc = bass.Bass("TRN2", target_bir_lowering=False, debug=False,
                   num_devices=CORES)

    # xm rides at the head of the ab tensor: one transfer (xm+ab span0)
    # gates the first mul instead of two serialized completions
    ab_d = nc.dram_tensor("ab", [P, T * 224 + T * 3584], b16,
                          kind="ExternalInput").ap()
    de_d = nc.dram_tensor("de", [P, T * 3584], b16, kind="ExternalInput").ap()
    m1_d = nc.dram_tensor("m1", [P, T * 112], b16, kind="ExternalInput").ap()
    w_d = nc.dram_tensor("w", [P, 2 * GP], b16, kind="ExternalInput").ap()
    o_d = nc.dram_tensor("o", [P, 2], f32, kind="ExternalOutput").ap()

    F2R = 2 * T * 224  # 3136: [t, m, g, k, r2] per-partition layout
    F1 = T * 224       # 1568

    with tile.TileContext(nc) as tc:
        with (
            tc.tile_pool(name="big", bufs=1) as big,
            tc.tile_pool(name="work", bufs=1) as work,
        ):
            ab_all = big.tile([P, T * 224 + T * 3584], b16)
            xm_sb = ab_all[:, 0:T * 224]
            XO = T * 224            # ab data offset within ab_all
            w_sb = big.tile([P, 2 * GP], b16)
            # ct2[:, 0:3136] = cd pairs (D|E colsums), [:, 3136:] = t pairs
            ct2 = big.tile([P, 2 * F2R], b16)
            vv = big.tile([P, F1], b16)     # [t, sel, g, k]: tn | m1
            cdf = big.tile([P, F1], b16)    # [t, m, g, k]: cd | ce
            pr = big.tile([P, F1], b16)
            tn2 = big.tile([P, F1], b16)    # m-summed t pairs

            des = []
            for si, (t0, ts) in enumerate(SPANS):
                E0, EN = t0 * 3584, ts * 3584
                de = work.tile([P, 2 * 3584], b16, tag="de", bufs=3)
                des.append(de)
                if si == 0:
                    nc.sync.dma_start(out=ab_all[:, 0:XO + EN],
                                      in_=ab_d[:, 0:XO + EN])
                else:
                    nc.sync.dma_start(out=ab_all[:, XO + E0:XO + E0 + EN],
                                      in_=ab_d[:, XO + E0:XO + E0 + EN])
                nc.sync.dma_start(out=de[:, 0:EN], in_=de_d[:, E0:E0 + EN])
            # epilogue-only inputs last: m1 into its vv slot (sel=1), w
            nc.sync.dma_start(
                out=vv[:, :].rearrange("p (t s x) -> p t s x",
                                       t=T, s=2, x=112)[:, :, 1],
                in_=m1_d.rearrange("p (t x) -> p t x", t=T, x=112))
            nc.sync.dma_start(out=w_sb[:, :], in_=w_d)

            t4 = work.tile([P, 2 * 1792], b16)
            for si, (t0, ts) in enumerate(SPANS):
                de = des[si]
                EN = ts * 3584
                U = ts * 14          # (t, m, g) groups in span
                tmp = work.tile([P, 2 * 3584], b16, tag="tmp", bufs=2)
                t8 = work.tile([P, 2 * 3584], b16, tag="t8", bufs=2)
                gjk = lambda ap: ap.rearrange("p (u j k) -> p u j k",
                                              u=U, j=16, k=16)
                xm_bc = (xm_sb[:, t0 * 224:(t0 + ts) * 224]
                         .rearrange("p (u k) -> p u k", u=U, k=16)
                         .unsqueeze(2).broadcast_to((P, U, 16, 16)))
                # fold 16->8: de half and tmp half -> adjacent halves of t8;
                # de-fold emitted first (only needs the de transfer)
                HV = ts * 224        # 16-wide rows per half
                d16 = de[:, 0:EN].rearrange("p (v k) -> p v k", v=HV, k=16)
                a16 = tmp[:, 0:EN].rearrange("p (v k) -> p v k", v=HV, k=16)
                e8d = t8[:, 0:HV * 8].rearrange("p (v k) -> p v k",
                                                v=HV, k=8)
                e8a = t8[:, HV * 8:HV * 16].rearrange("p (v k) -> p v k",
                                                      v=HV, k=8)
                nc.vector.tensor_mul(out=gjk(tmp[:, 0:EN]),
                                     in0=gjk(ab_all[:, XO + t0 * 3584:
                                                    XO + (t0 + ts) * 3584]),
                                     in1=xm_bc)
                nc.vector.tensor_add(out=e8a, in0=a16[:, :, 0:8],
                                     in1=a16[:, :, 8:16])
                nc.vector.tensor_add(out=e8d, in0=d16[:, :, 0:8],
                                     in1=d16[:, :, 8:16])
                # merged 8->4 over [de | tmp]
                V = ts * 448
                e8 = t8[:, 0:V * 8].rearrange("p (v k) -> p v k", v=V, k=8)
                e4 = t4[:, 0:V * 4].rearrange("p (v k) -> p v k", v=V, k=4)
                nc.vector.tensor_add(out=e4, in0=e8[:, :, 0:4],
                                     in1=e8[:, :, 4:8])
                # merged 4->2, pairs kept: -> two segments of ct2
                ct_v = (ct2[:, :].rearrange("p (c f) -> p c f", c=2, f=F2R)
                        [:, :, t0 * 448:(t0 + ts) * 448]
                        .rearrange("p c (y r) -> p c y r",
                                   y=ts * 224, r=2))
                nc.vector.tensor_add(
                    out=ct_v,
                    in0=e4[:, :, 0:2].rearrange("p (c y) r -> p c y r",
                                                c=2, y=ts * 224),
                    in1=e4[:, :, 2:4].rearrange("p (c y) r -> p c y r",
                                                c=2, y=ts * 224))

            # ---- epilogue ----
            cd2 = ct2[:, 0:F2R]                  # [t, m, g, k, r] pairs
            t12 = ct2[:, F2R:2 * F2R]
            tmx = lambda ap: ap.rearrange("p (t m x) -> p t m x",
                                          t=T, m=2, x=224)
            # tn2[t,g,k,r] = t12[t,0,..] + t12[t,1,..]   (m-sum, 2x)
            tn2h = tn2[:, 0:F1].rearrange("p (t x) -> p t x", t=T, x=224)
            nc.vector.tensor_add(out=tn2h, in0=tmx(t12)[:, :, 0],
                                 in1=tmx(t12)[:, :, 1])
            # collapse pairs (1x, small): tn -> vv[sel=0]; cd2 -> cdf
            tr = tn2[:, 0:F1].rearrange("p (v r) -> p v r", v=F1 // 2, r=2)
            nc.vector.tensor_add(
                out=vv[:, :].rearrange("p (t s x) -> p t s x",
                                       t=T, s=2, x=112)[:, :, 0],
                in0=tr[:, :, 0].rearrange("p (t x) -> p t x", t=T, x=112),
                in1=tr[:, :, 1].rearrange("p (t x) -> p t x", t=T, x=112))
            cr = cd2.rearrange("p (v r) -> p v r", v=F2R // 2, r=2)
            nc.vector.tensor_add(out=cdf[:, :], in0=cr[:, :, 0],
                                 in1=cr[:, :, 1])
            # pr[t,m,g,k] = cdf * (tn | m1)
            nc.vector.tensor_mul(out=pr[:, :], in0=cdf[:, :], in1=vv[:, :])
            # fold [98, 16] -> [98]
            p16 = pr[:, :].rearrange("p (v k) -> p v k", v=98, k=16)
            h8 = tn2[:, 0:784].rearrange("p (v k) -> p v k", v=98, k=8)
            nc.vector.tensor_add(out=h8, in0=p16[:, :, 0:8],
                                 in1=p16[:, :, 8:16])
            h4 = tn2[:, 784:1176].rearrange("p (v k) -> p v k", v=98, k=4)
            nc.vector.tensor_add(out=h4, in0=h8[:, :, 0:4], in1=h8[:, :, 4:8])
            h2 = tn2[:, 1176:1372].rearrange("p (v k) -> p v k", v=98, k=2)
            nc.vector.tensor_add(out=h2, in0=h4[:, :, 0:2], in1=h4[:, :, 2:4])
            h1 = tn2[:, 1372:1470]
            nc.vector.tensor_add(out=h1, in0=h2[:, :, 0], in1=h2[:, :, 1])
            # s[t,g] = h1[t,0,g] + h1[t,1,g]
            s_all = tn2[:, 1470:1519]
            gm = h1.rearrange("p (t m g) -> p t m g", t=T, m=2, g=S)
            nc.vector.tensor_add(out=s_all.rearrange("p (t g) -> p t g",
                                                     t=T, g=S),
                                 in0=gm[:, :, 0], in1=gm[:, :, 1])
            # head: hm[c, tg] = s[tg] * w[c, tg]; o[c] = sum_tg hm
            hm = pr[:, 0:2 * GP].rearrange("p (c q) -> p c q", c=2, q=GP)
            nc.vector.tensor_mul(
                out=hm,
                in0=w_sb[:, :].rearrange("p (c q) -> p c q", c=2, q=GP),
                in1=s_all.unsqueeze(1).broadcast_to((P, 2, GP)))
            o_sb = big.tile([P, 2], f32)
            nc.vector.reduce_sum(out=o_sb[:, :].rearrange("p c -> p c"),
                                 in_=hm, axis=mybir.AxisListType.X)
            nc.sync.dma_start(out=o_d, in_=o_sb[:, :])

    return nc


def _get_nc():
    if "nc" not in _cache:
        _cache["nc"] = _build_nc()
        import concourse.mybir as mybir
        _split_multiwait(_cache["nc"], mybir)
    return _cache["nc"]


def kernel(h0, cw0, mw0, cw1, mw1,
           msg0_r0, msg0_r1, msg0_r2,
           msg1_r0, msg1_r1, msg1_r2,
           w_pred, b_pred):
    from concourse.bass_utils import run_bass_kernel_spmd

    nc = _get_nc()

    def pad_mat(m):
        out = np.zeros((NP, 256), np.float32)
        out[:N] = np.asarray(m, np.float32).reshape(N, 256)
        return out.reshape(CORES, T, 128, S, 16, 16)  # [c,t,p,g,j,k]

    A5 = pad_mat(cw0[0])
    B5 = pad_mat(mw0[0])
    # AB: [c,t,p, m,g,j,k] -> (c,t,p,3584)
    AB = np.ascontiguousarray(
        np.stack([A5, B5], axis=3).reshape(CORES, T, 128, 3584)
        .transpose(0, 2, 1, 3).reshape(CORES, 128, T * 3584)).astype(BF16)

    D5 = pad_mat(cw1[0])
    E5 = pad_mat(mw1[0])
    # DE: j innermost for the fold tree: [c,t,p, m,g,k,j] -> (c,t,p,3584)
    DE = np.ascontiguousarray(
        np.stack([D5.transpose(0, 1, 2, 3, 5, 4),
                  E5.transpose(0, 1, 2, 3, 5, 4)], axis=3)
        .reshape(CORES, T, 128, 3584)
        .transpose(0, 2, 1, 3).reshape(CORES, 128, T * 3584)).astype(BF16)

    def pad_vec(v):
        out = np.zeros((NP, 16), np.float32)
        out[:N] = np.asarray(v, np.float32).reshape(N, 16)
        return out.reshape(CORES, T, 128, S, 16)

    X = pad_vec(np.asarray(h0, np.float32)[..., 0])
    M0 = pad_vec(np.asarray(msg0_r0, np.float32)[..., 0])
    XM = (np.stack([X, M0], axis=3).reshape(CORES, T, 128, 224)
          .transpose(0, 2, 1, 3).reshape(CORES, 128, T * 224)).astype(BF16)
    AB = np.ascontiguousarray(np.concatenate([XM, AB], axis=2))
    M1 = np.ascontiguousarray(
        pad_vec(np.asarray(msg1_r0, np.float32)[..., 0])
        .reshape(CORES, T, 128, 112)
        .transpose(0, 2, 1, 3).reshape(CORES, 128, T * 112)).astype(BF16)

    wp = np.zeros((2, NP), np.float32)
    wp[:, :N] = np.asarray(w_pred, np.float32)
    W = np.ascontiguousarray(
        wp.reshape(2, CORES, T, 128, S).transpose(1, 3, 0, 2, 4)
        .reshape(CORES, 128, 2 * GP)).astype(BF16)

    in_maps = [
        {"ab": AB[i], "de": DE[i], "m1": M1[i], "w": W[i]}
        for i in range(CORES)
    ]
    res = run_bass_kernel_spmd(nc, in_maps, list(range(CORES)), trace=TRACE)
    _cache["last_res"] = res
    partial = np.zeros(2, np.float64)
    for i in range(CORES):
        partial += res.results[i]["o"].astype(np.float64).sum(axis=0)
    out = (partial + np.asarray(b_pred, np.float64)).astype(np.float32)
    return out.reshape(1, 2)
